# revision 18
# baseline (speedup 1.0000x reference)
"""DeepSeekMoE layer on 8 Trainium2 NeuronCores.

Problem (hardcoded): B=2, T=1024, C=1024, H=4096, E_routed=8 (top-2 sigmoid
gating), E_shared=2, fp32 in/out.

Sharding (expert-parallel, all tokens per core):
  Core c: routed expert r=c on ALL 2048 tokens (gate-masked dense), plus a
  1024-wide H-slice (hs=c%4) of shared expert s=c//4 on ALL 2048 tokens.
  Device emits two bf16 partial tensors [128, CK*2048] (c-tile-major,
  transposed); host sums all 16 partials + u.

Why this layout: each stationary weight tile feeds 4 matmuls (4 token
blocks of 512), and a post-legalize pass shrinks the 3 duplicate
InstLdweights per tile to 1-column idempotent reloads — LDWEIGHTS time was
measured fully serial with matmuls (~53ns/load bf16-FWL), so amortizing it
4x is worth ~100us/core.  All expert matmuls run in bf16 (2 cols/cycle).

loop_m > 1 wraps the body in a hardware For_i loop for wall-clock timing.
"""
import contextlib
import dataclasses
import os
import numpy as np

import concourse.bass as bass
import concourse.tile as tile
from concourse import bacc, mybir
from concourse import bass_utils
from concourse.alu_op_type import AluOpType
from concourse.masks import make_identity

F32 = mybir.dt.float32
F32R = mybir.dt.float32r
BF16 = mybir.dt.bfloat16
AF = mybir.ActivationFunctionType
AX = mybir.AxisListType

B, T, C, H = 2, 1024, 1024, 4096
E_R, E_S = 8, 2
NCORES = 8
NTOK = 2048         # tokens per core (all of them)
TB = 4              # token blocks
TOKB = 512          # tokens per block
CK = C // 128       # 8 c-tiles
HK = H // 128       # 32 h-tiles (routed); shared slice has 8
HSK = 8             # h-tiles in the shared expert slice
EPS = 1.1920929e-07
NEG = -1e30

_CACHE = {}


def _shrink_dup_ldweights(nc):
    """Replace consecutive duplicate InstLdweights with a 1-column reload of
    the same data: idempotent on the PE array, ~50x cheaper, and keeps the
    walrus LDW<->matmult pairing intact (deleting the instruction breaks it).
    """
    n = 0
    for b in nc.main_func.blocks:
        last = None
        for i in b.instructions:
            if isinstance(i, mybir.InstLdweights):
                key = (str(i.ins[0]), str(i.tile_size), str(i.tile_position),
                       str(i.perf_mode), str(i.is_transpose))
                si = i.sync_info
                clean = si is None or (
                    len(si.on_wait) == 0 and len(si.on_update) == 0)
                if key == last and clean:
                    bap = i.ins[0].bass_ap
                    pairs = [list(p) for p in bap.ap]
                    for d in range(1, len(pairs)):
                        pairs[d][1] = 1
                    nb = dataclasses.replace(bap, ap=type(bap.ap)(pairs))
                    i.ins = [nc.tensor.lower_ap(nb, opt=False,
                                                for_matmul_weights=True)]
                    n += 1
                else:
                    last = key
    return n


def _build_program(loop_m=1):
    nc = bacc.Bacc("TRN2", target_bir_lowering=False, debug=False,
                   enable_asserts=False)

    d = {}
    d["uTr"] = nc.dram_tensor("uTr", [128, CK * NTOK], F32R, kind="ExternalInput").ap()
    d["w1"] = nc.dram_tensor("w1", [HK, 128, CK * 128], BF16, kind="ExternalInput").ap()
    d["w2"] = nc.dram_tensor("w2", [CK, 2, 128, 16 * 128], BF16, kind="ExternalInput").ap()
    d["w1s"] = nc.dram_tensor("w1s", [HSK, 128, CK * 128], BF16, kind="ExternalInput").ap()
    d["w2s"] = nc.dram_tensor("w2s", [CK, 128, HSK * 128], BF16, kind="ExternalInput").ap()
    d["b1"] = nc.dram_tensor("b1t", [128, HK + HSK], F32, kind="ExternalInput").ap()
    d["b2r"] = nc.dram_tensor("b2r", [128, CK], F32, kind="ExternalInput").ap()
    d["b2s"] = nc.dram_tensor("b2s", [128, CK], F32, kind="ExternalInput").ap()
    d["cen"] = nc.dram_tensor("cen", [128, CK * E_R], F32R, kind="ExternalInput").ap()
    d["esel"] = nc.dram_tensor("esel", [E_R, 128], F32R, kind="ExternalInput").ap()
    d["outR"] = nc.dram_tensor("outR", [128, CK * NTOK], BF16, kind="ExternalOutput").ap()
    d["outS"] = nc.dram_tensor("outS", [128, CK * NTOK], BF16, kind="ExternalOutput").ap()

    with tile.TileContext(nc) as tc:
        with (
            tc.tile_pool(name="cst", bufs=1) as cst,
            tc.tile_pool(name="io", bufs=1) as io,
            tc.tile_pool(name="wk", bufs=1) as wk,
            tc.tile_pool(name="up", bufs=2) as up,
            tc.tile_pool(name="w1p", bufs=2) as w1p,
            tc.tile_pool(name="w2p", bufs=2) as w2p,
            tc.tile_pool(name="hp", bufs=32) as hp,
            tc.tile_pool(name="sg", bufs=2) as sg,
            tc.tile_pool(name="pp", bufs=8, space="PSUM") as pp,
        ):
            pools = dict(cst=cst, io=io, wk=wk, up=up, w1p=w1p, w2p=w2p,
                         hp=hp, sg=sg, pp=pp)
            loop = tc.For_i(0, loop_m, 1) if loop_m > 1 else contextlib.nullcontext()
            with loop:
                _moe_body(nc, d, pools)

    nshrunk = _shrink_dup_ldweights(nc)
    nc.compile()
    nc._moe_nshrunk = nshrunk
    return nc


def _moe_body(nc, d, p):
    cst, io, wk, up, w1p, w2p, hp, sg, pp = (
        p["cst"], p["io"], p["wk"], p["up"], p["w1p"], p["w2p"], p["hp"],
        p["sg"], p["pp"])

    # ---- constants ----
    ident = cst.tile([128, 128], F32, tag="ident", name="ident")
    make_identity(nc, ident[:])
    ones_f = cst.tile([128, 1], F32, tag="ones_f", name="ones_f")
    nc.gpsimd.memset(ones_f[:], 1.0)
    onescol = cst.tile([128, 1], F32R, tag="onescol", name="onescol")
    nc.vector.tensor_copy(onescol[:], ones_f[:])
    onesrow_f = cst.tile([1, 128], F32, tag="onesrow_f", name="onesrow_f")
    nc.gpsimd.memset(onesrow_f[:], 1.0)
    onesrow = cst.tile([1, 128], F32R, tag="onesrow", name="onesrow")
    nc.vector.tensor_copy(onesrow[:], onesrow_f[:])
    epsb = cst.tile([1, 1], F32, tag="epsb", name="epsb")
    nc.gpsimd.memset(epsb[:], EPS)
    sclb = cst.tile([1, 1], F32, tag="sclb", name="sclb")
    nc.gpsimd.memset(sclb[:], 1.0 / C)
    negb = cst.tile([128, 1], F32, tag="negb", name="negb")
    nc.gpsimd.memset(negb[:], NEG)

    cen = io.tile([128, CK * E_R], F32R, tag="cen", name="cen")
    nc.sync.dma_start(cen[:], d["cen"])
    esel = io.tile([E_R, 128], F32R, tag="esel", name="esel")
    nc.sync.dma_start(esel[:], d["esel"])
    b1 = io.tile([128, HK + HSK], F32, tag="b1", name="b1")
    nc.sync.dma_start(b1[:], d["b1"])
    b2r = io.tile([128, CK], F32, tag="b2r", name="b2r")
    nc.sync.dma_start(b2r[:], d["b2r"])
    b2s = io.tile([128, CK], F32, tag="b2s", name="b2s")
    nc.sync.dma_start(b2s[:], d["b2s"])

    def tsl(tb):
        return slice(TOKB * tb, TOKB * (tb + 1))

    # ---- pass A: stream u k-tile halves; rms stats + routing scores ----
    ss_ps = [pp.tile([1, TOKB], F32, tag="pp", name=f"ss{tb}") for tb in range(TB)]
    sc_ps = [pp.tile([E_R, TOKB], F32, tag="pp", name=f"sc{tb}") for tb in range(TB)]
    for k in range(CK):
        for tb in range(TB):
            uk = up.tile([128, TOKB], F32R, tag="uk", name=f"uk{k}_{tb}")
            off = NTOK * k + TOKB * tb
            nc.sync.dma_start(uk[:], d["uTr"][:, off:off + TOKB])
            usq = wk.tile([128, TOKB], F32R, tag="usq", bufs=1,
                          name=f"usq{k}_{tb}")
            nc.vector.tensor_mul(usq[:], uk[:], uk[:])
            nc.tensor.matmul(ss_ps[tb][:], onescol[:], usq[:],
                             start=(k == 0), stop=(k == CK - 1))
            nc.tensor.matmul(sc_ps[tb][:], cen[:, E_R * k:E_R * (k + 1)],
                             uk[:],
                             start=(k == 0), stop=(k == CK - 1))

    # invrms (scalar part only; broadcast happens after gating)
    invrs = []
    for tb in range(TB):
        rms = wk.tile([1, TOKB], F32, tag="rms", bufs=2, name=f"rms{tb}")
        nc.scalar.activation(rms[:], ss_ps[tb][:], AF.Sqrt, bias=epsb[:], scale=sclb[:])
        invr = wk.tile([1, TOKB], F32R, tag="invr", bufs=4, name=f"invr{tb}")
        with nc.allow_low_precision(reason="invrms feeds a f32r matmul"):
            nc.vector.reciprocal(invr[:], rms[:])
        invrs.append(invr)

    # ---- gating per token block: top-2 sigmoid on 128-token chunks ----
    wrep = []
    for tb in range(TB):
        scT = wk.tile([E_R, TOKB], F32, tag="scT", bufs=1, name=f"scT{tb}")
        nc.vector.tensor_copy(scT[:], sc_ps[tb][:])
        gT = wk.tile([E_R, TOKB], F32R, tag="gT", bufs=1, name=f"gT{tb}")
        for tt in range(TOKB // 128):
            blk = slice(128 * tt, 128 * (tt + 1))
            tr_ps = pp.tile([128, E_R], F32, tag="pp", name=f"tr{tb}_{tt}")
            nc.tensor.matmul(tr_ps[:], scT[:, blk], ident[:E_R, :E_R],
                             start=True, stop=True)
            sig = wk.tile([128, E_R], F32, tag="sig", bufs=2, name=f"sig{tb}_{tt}")
            den = wk.tile([128, 1], F32, tag="den", bufs=2, name=f"den{tb}_{tt}")
            nc.scalar.activation(sig[:], tr_ps[:], AF.Sigmoid, accum_out=den[:])
            invd = wk.tile([128, 1], F32, tag="invd", bufs=2, name=f"invd{tb}_{tt}")
            nc.vector.reciprocal(invd[:], den[:])
            m1 = wk.tile([128, 1], F32, tag="m1", bufs=2, name=f"m1_{tb}_{tt}")
            nc.vector.reduce_max(m1[:], sig[:], axis=AX.X)
            mk1 = wk.tile([128, E_R], F32, tag="mk1", bufs=2, name=f"mk1_{tb}_{tt}")
            nc.vector.tensor_scalar(mk1[:], sig[:], m1[:], None, AluOpType.is_ge)
            s2 = wk.tile([128, E_R], F32, tag="s2", bufs=2, name=f"s2_{tb}_{tt}")
            nc.vector.scalar_tensor_tensor(s2[:], mk1[:], negb[:], sig[:],
                                           AluOpType.mult, AluOpType.add)
            m2 = wk.tile([128, 1], F32, tag="m2", bufs=2, name=f"m2_{tb}_{tt}")
            nc.vector.reduce_max(m2[:], s2[:], axis=AX.X)
            mk = wk.tile([128, E_R], F32, tag="mk", bufs=2, name=f"mk_{tb}_{tt}")
            nc.vector.tensor_scalar(mk[:], sig[:], m2[:], None, AluOpType.is_ge)
            gsel = wk.tile([128, E_R], F32, tag="gsel", bufs=2, name=f"gsel{tb}_{tt}")
            nc.vector.tensor_mul(gsel[:], sig[:], mk[:])
            gt8 = wk.tile([128, E_R], F32, tag="gt8", bufs=2, name=f"gt8_{tb}_{tt}")
            nc.vector.tensor_scalar_mul(gt8[:], gsel[:], invd[:])
            bk_ps = pp.tile([E_R, 128], F32, tag="pp", name=f"bk{tb}_{tt}")
            nc.tensor.transpose(bk_ps[:], gt8[:], ident[:])
            nc.vector.tensor_copy(gT[:, blk], bk_ps[:])
        # this core's gate row broadcast to 128 partitions
        wr_ps = pp.tile([128, TOKB], F32, tag="pp", name=f"wr{tb}")
        nc.tensor.matmul(wr_ps[:], esel[:], gT[:], start=True, stop=True)
        wr = sg.tile([128, TOKB], BF16, tag=f"wrep{tb}", bufs=1, name=f"wrep{tb}")
        nc.vector.tensor_copy(wr[:], wr_ps[:])
        wrep.append(wr)

    # invrms broadcast to [128, TOKB] (after gating so its PSUM slots are
    # free by the time the W1 rotation reaches them)
    ir_ps = []
    for tb in range(TB):
        irp = pp.tile([128, TOKB], F32, tag="pp", name=f"ir{tb}")
        nc.tensor.matmul(irp[:], onesrow[:], invrs[tb][:], start=True, stop=True)
        ir_ps.append(irp)

    # ---- pass B: re-stream u, xn = u * invrms  (bf16, resident) ----
    xns = []
    for k in range(CK):
        xs = io.tile([128, NTOK], BF16, tag=f"xns{k}", name=f"xns{k}")
        for tb in range(TB):
            ukb = up.tile([128, TOKB], F32R, tag="uk", name=f"ukb{k}_{tb}")
            off = NTOK * k + TOKB * tb
            nc.sync.dma_start(ukb[:], d["uTr"][:, off:off + TOKB])
            nc.vector.tensor_tensor(xs[:, tsl(tb)], ukb[:],
                                    ir_ps[tb][:], AluOpType.mult)
        xns.append(xs)

    def mlp(w1d, w2d, nhk, b1off, stage_to, evict):
        """W1 (nhk strips) -> gelu h -> W2 (CK c-tiles) -> evict(ct, tb, py)."""
        hts = []
        for hh in range(nhk):
            w1c = w1p.tile([128, CK * 128], BF16, tag="w1c", name=f"w1c_{b1off}_{hh}")
            nc.sync.dma_start(w1c[:], w1d[hh])
            ph = [pp.tile([128, TOKB], F32, tag="pp", name=f"ph_{b1off}_{hh}_{tb}")
                  for tb in range(TB)]
            for k in range(CK):
                for tb in range(TB):
                    nc.tensor.matmul(ph[tb][:], w1c[:, 128 * k:128 * (k + 1)],
                                     xns[k][:, tsl(tb)],
                                     start=(k == 0), stop=(k == CK - 1))
            ht = hp.tile([128, NTOK], BF16, tag="h", name=f"h_{b1off}_{hh}")
            for tb in range(TB):
                nc.scalar.activation(ht[:, tsl(tb)], ph[tb][:], AF.Gelu,
                                     bias=b1[:, b1off + hh:b1off + hh + 1])
            hts.append(ht)
        for ct in range(CK):
            py = [pp.tile([128, TOKB], F32, tag="pp", name=f"py_{b1off}_{ct}_{tb}")
                  for tb in range(TB)]
            if nhk == HK:
                for half in range(2):
                    for qq in range(2):
                        w2c = w2p.tile([128, 8 * 128], BF16, tag="w2c",
                                       name=f"w2c_{ct}_{half}_{qq}")
                        nc.sync.dma_start(
                            w2c[:], w2d[ct, half][:, 1024 * qq:1024 * (qq + 1)])
                        for kk in range(8):
                            gk = 16 * half + 8 * qq + kk
                            for tb in range(TB):
                                nc.tensor.matmul(
                                    py[tb][:], w2c[:, 128 * kk:128 * (kk + 1)],
                                    hts[gk][:, tsl(tb)],
                                    start=(gk == 0), stop=(gk == HK - 1))
            else:
                w2c = w2p.tile([128, HSK * 128], BF16, tag="w2sc",
                               name=f"w2sc_{ct}")
                nc.sync.dma_start(w2c[:], w2d[ct])
                for kk in range(HSK):
                    for tb in range(TB):
                        nc.tensor.matmul(
                            py[tb][:], w2c[:, 128 * kk:128 * (kk + 1)],
                            hts[kk][:, tsl(tb)],
                            start=(kk == 0), stop=(kk == HSK - 1))
            for tb in range(TB):
                st = sg.tile([128, TOKB], BF16, tag="stage", bufs=2,
                             name=f"st_{b1off}_{ct}_{tb}")
                evict(st[:], py[tb][:], ct, tb)
                off = NTOK * ct + TOKB * tb
                nc.sync.dma_start(stage_to[:, off:off + TOKB], st[:])

    # routed expert: out = (W2h + b2r) * gate
    def evict_r(dst, py, ct, tb):
        nc.vector.scalar_tensor_tensor(dst, py, b2r[:, ct:ct + 1], wrep[tb][:],
                                       AluOpType.add, AluOpType.mult)

    # shared expert slice: out = W2h + b2s (b2s zeroed on host for hs != 0)
    def evict_s(dst, py, ct, tb):
        nc.vector.tensor_scalar(dst, py, b2s[:, ct:ct + 1], None, AluOpType.add)

    mlp(d["w1"], d["w2"], HK, 0, d["outR"], evict_r)
    mlp(d["w1s"], d["w2s"], HSK, HK, d["outS"], evict_s)


def _prep_inputs(u, g_shared, W1_s, b1_s, W2_s, b2_s,
                 g_routed, W1_r, b1_r, W2_r, b2_r, centroids):
    f = np.float32
    bf = mybir.dt.np(BF16)
    u2 = np.ascontiguousarray(np.asarray(u, f).reshape(B * T, C))
    # uT[p, 2048k + t] = u2[t, 128k + p]  (same for all cores)
    uT = np.ascontiguousarray(
        u2.T.reshape(CK, 128, NTOK).transpose(1, 0, 2)).reshape(128, CK * NTOK)
    cenT = np.ascontiguousarray(
        np.asarray(centroids, f).reshape(CK, 128, E_R).transpose(1, 0, 2)
    ).reshape(128, CK * E_R)
    gsh = np.asarray(g_shared, f).reshape(C, 1)
    grt = np.asarray(g_routed, f).reshape(C, 1)

    in_maps = []
    for c in range(NCORES):
        s, hs = c // 4, c % 4
        W1g = np.asarray(W1_r[c], f) * grt                 # [C, H]
        W2g = np.asarray(W2_r[c], f)                       # [H, C]
        # w1[hh][p, 128k + j] = W1g[128k+p, 128hh + j]
        w1h = np.ascontiguousarray(
            W1g.reshape(CK, 128, HK, 128).transpose(2, 1, 0, 3)
        ).reshape(HK, 128, CK * 128).astype(bf)
        # w2[ct, half][p, 128kk + j] = W2g[128*(16half+kk)+p, 128ct + j]
        w2h = np.ascontiguousarray(
            W2g.reshape(2, 16, 128, CK, 128).transpose(3, 0, 2, 1, 4)
        ).reshape(CK, 2, 128, 16 * 128).astype(bf)
        # shared: H-slice [1024hs : 1024(hs+1)]
        W1sg = (np.asarray(W1_s[s], f) * gsh)[:, 1024 * hs:1024 * (hs + 1)]
        W2sg = np.asarray(W2_s[s], f)[1024 * hs:1024 * (hs + 1), :]
        w1sh = np.ascontiguousarray(
            W1sg.reshape(CK, 128, HSK, 128).transpose(2, 1, 0, 3)
        ).reshape(HSK, 128, CK * 128).astype(bf)
        w2sh = np.ascontiguousarray(
            W2sg.reshape(HSK, 128, CK, 128).transpose(2, 1, 0, 3)
        ).reshape(CK, 128, HSK * 128).astype(bf)
        # b1t: [128, HK + HSK]
        b1t = np.empty((128, HK + HSK), f)
        b1t[:, :HK] = np.asarray(b1_r[c], f).reshape(HK, 128).T
        b1t[:, HK:] = np.asarray(
            b1_s[s], f)[1024 * hs:1024 * (hs + 1)].reshape(HSK, 128).T
        b2rt = np.ascontiguousarray(np.asarray(b2_r[c], f).reshape(CK, 128).T)
        if hs == 0:
            b2st = np.ascontiguousarray(np.asarray(b2_s[s], f).reshape(CK, 128).T)
        else:
            b2st = np.zeros((128, CK), f)
        es = np.zeros((E_R, 128), f)
        es[c, :] = 1.0
        in_maps.append({
            "uTr": uT, "w1": w1h, "w2": w2h, "w1s": w1sh, "w2s": w2sh,
            "b1t": b1t, "b2r": b2rt, "b2s": b2st, "cen": cenT, "esel": es,
        })
    return in_maps, u2


def _run(in_maps, trace=False):
    if "nc" not in _CACHE:
        _CACHE["nc"] = _build_program()
    nc = _CACHE["nc"]
    res = bass_utils.run_bass_kernel_spmd(
        nc, in_maps, core_ids=list(range(NCORES)), trace=trace)
    return res


def kernel(**inputs):
    in_maps, u2 = _prep_inputs(**inputs)
    trace = bool(int(os.environ.get("MOE_TRACE", "0")))
    res = _run(in_maps, trace=trace)
    _CACHE["last_results"] = res
    acc = u2.astype(np.float64)
    for c in range(NCORES):
        for nm in ("outR", "outS"):
            part = res.results[c][nm].astype(np.float32)
            part = part.reshape(128, CK, NTOK).transpose(2, 1, 0).reshape(NTOK, C)
            acc += part
    return acc.astype(np.float32).reshape(B, T, C)


# revision 19
# speedup vs baseline: 1.1548x; 1.1548x over previous
"""DeepSeekMoE layer on 8 Trainium2 NeuronCores.

Problem (hardcoded): B=2, T=1024, C=1024, H=4096, E_routed=8 (top-2 sigmoid
gating), E_shared=2, fp32.

Sharding: 2-way expert-parallel x 4-way token-parallel.
  Core c (g = c//4, q = c%4) processes token quarter q (512 tokens) for the
  expert set {shared[g], routed[4g], .., routed[4g+3]} and emits the partial
  sum of those 5 expert contributions in transposed layout [C, 512].
  Host: out[q] = partial[q] + partial[q+4] + u[q]  (residual on host).

Device kernel (per core, SPMD — identical program, different data):
  T-layout throughout: activations [C-partition, token-free], tokens N=512.
  rmsnorm stats via squared tiles + ones-matmul column reduce; top-2 sigmoid
  gating in token-layout, transposed via PE, broadcast via one-hot matmuls.
  Expert MLP: W1 stationary [128,128] tiles x xnT moving (N=512) -> PSUM ->
  gelu(+b1) on ACT -> h_act -> W2 stationary x h_act moving -> PSUM
  (with b2 folded in via a K=1 matmul) -> eviction (gate mult + add) on DVE
  into the SBUF accumulator.  All expert matmuls run in bf16 (weights and
  activations): the PE streams 2 bf16 moving columns/cycle (vs 1 for f32r)
  and FWL halves the LDWEIGHTS time, measured ~1.9x faster than the f32r
  version at rel-L2 ~2e-3 (tolerance 2e-2).

loop_m > 1 wraps the whole body in a hardware For_i loop — used only for
wall-clock timing (difference M vs 1 iterations to cancel dispatch overhead).
"""
import contextlib
import os
import numpy as np

import concourse.bass as bass
import concourse.tile as tile
from concourse import bacc, mybir
from concourse import bass_utils
from concourse.alu_op_type import AluOpType
from concourse.masks import make_identity

F32 = mybir.dt.float32
F32R = mybir.dt.float32r
BF16 = mybir.dt.bfloat16
AF = mybir.ActivationFunctionType
AX = mybir.AxisListType

B, T, C, H = 2, 1024, 1024, 4096
E_R, E_S = 8, 2
NCORES = 8
TOKC = 512          # tokens per core
CK = C // 128       # 8 c-tiles
HK = H // 128       # 32 h-tiles
NMC = 16            # W1 m-chunks (each 2 h-tiles = 256 h cols)
NE = 5              # expert passes per core: 1 shared + 4 routed
EPS = 1.1920929e-07
NEG = -1e30

_CACHE = {}


def _build_program(loop_m=1):
    nc = bacc.Bacc("TRN2", target_bir_lowering=False, debug=False,
                   enable_asserts=False)

    d = {}
    d["uT"] = nc.dram_tensor("uT", [128, CK * TOKC], F32, kind="ExternalInput").ap()
    d["uTr"] = nc.dram_tensor("uTr", [128, CK * TOKC], F32R, kind="ExternalInput").ap()
    d["w1"] = nc.dram_tensor("w1", [NE, NMC, 128, CK * 256], BF16, kind="ExternalInput").ap()
    d["w2"] = nc.dram_tensor("w2", [NE, HK // 2, 128, 2 * 1024], BF16, kind="ExternalInput").ap()
    d["b1"] = nc.dram_tensor("b1t", [128, NE * HK], F32, kind="ExternalInput").ap()
    d["b2r"] = nc.dram_tensor("b2r", [1, NE * CK * 128], F32R, kind="ExternalInput").ap()
    d["cen"] = nc.dram_tensor("cen", [128, CK * E_R], F32R, kind="ExternalInput").ap()
    d["esel"] = nc.dram_tensor("esel", [E_R, 4 * 128], F32R, kind="ExternalInput").ap()
    d["out"] = nc.dram_tensor("outT", [128, CK * TOKC], F32, kind="ExternalOutput").ap()

    with tile.TileContext(nc) as tc:
        with (
            tc.tile_pool(name="cst", bufs=1) as cst,
            tc.tile_pool(name="io", bufs=1) as io,
            tc.tile_pool(name="wk", bufs=1) as wk,
            tc.tile_pool(name="w1p", bufs=2) as w1p,
            tc.tile_pool(name="w2p", bufs=3) as w2p,
            tc.tile_pool(name="hp", bufs=32) as hp,
            tc.tile_pool(name="pp", bufs=8, space="PSUM") as pp,
        ):
            pools = dict(cst=cst, io=io, wk=wk, w1p=w1p, w2p=w2p, hp=hp, pp=pp)
            loop = tc.For_i(0, loop_m, 1) if loop_m > 1 else contextlib.nullcontext()
            with loop:
                _moe_body(nc, d, pools)

    nc.compile()
    return nc


def _moe_body(nc, d, p):
    cst, io, wk, w1p, w2p, hp, pp = (
        p["cst"], p["io"], p["wk"], p["w1p"], p["w2p"], p["hp"], p["pp"])

    # ---- constants ----
    ident = cst.tile([128, 128], F32, tag="ident", name="ident")
    make_identity(nc, ident[:])
    ones_f = cst.tile([128, 1], F32, tag="ones_f", name="ones_f")
    nc.gpsimd.memset(ones_f[:], 1.0)
    onescol = cst.tile([128, 1], F32R, tag="onescol", name="onescol")
    nc.vector.tensor_copy(onescol[:], ones_f[:])
    ones512_f = cst.tile([1, TOKC], F32, tag="ones512_f", name="ones512_f")
    nc.gpsimd.memset(ones512_f[:], 1.0)
    ones512 = cst.tile([1, TOKC], F32R, tag="ones512", name="ones512")
    nc.vector.tensor_copy(ones512[:], ones512_f[:])
    onesrow_f = cst.tile([1, 128], F32, tag="onesrow_f", name="onesrow_f")
    nc.gpsimd.memset(onesrow_f[:], 1.0)
    onesrow = cst.tile([1, 128], F32R, tag="onesrow", name="onesrow")
    nc.vector.tensor_copy(onesrow[:], onesrow_f[:])
    epsb = cst.tile([1, 1], F32, tag="epsb", name="epsb")
    nc.gpsimd.memset(epsb[:], EPS)
    sclb = cst.tile([1, 1], F32, tag="sclb", name="sclb")
    nc.gpsimd.memset(sclb[:], 1.0 / C)
    negb = cst.tile([128, 1], F32, tag="negb", name="negb")
    nc.gpsimd.memset(negb[:], NEG)

    # ---- input loads (single DMA each) ----
    uT = io.tile([128, CK * TOKC], F32, tag="uT", name="uT")
    uTr = io.tile([128, CK * TOKC], F32R, tag="uTr", name="uTr")
    for k in range(CK):
        sl = slice(TOKC * k, TOKC * (k + 1))
        nc.sync.dma_start(uT[:, sl], d["uT"][:, sl])
        nc.sync.dma_start(uTr[:, sl], d["uTr"][:, sl])
    cen = io.tile([128, CK * E_R], F32R, tag="cen", name="cen")
    nc.sync.dma_start(cen[:], d["cen"])
    esel = io.tile([E_R, 4 * 128], F32R, tag="esel", name="esel")
    nc.sync.dma_start(esel[:], d["esel"])
    b1 = io.tile([128, NE * HK], F32, tag="b1", name="b1")
    nc.sync.dma_start(b1[:], d["b1"])

    def uslc(k):
        return slice(TOKC * k, TOKC * (k + 1))

    # ---- rmsnorm stats: invrms over all 512 tokens ----
    ss_ps = pp.tile([1, TOKC], F32, tag="pp", name="ss_ps")
    for k in range(CK):
        usq = wk.tile([128, TOKC], F32R, tag="usq", bufs=2, name=f"usq{k}")
        nc.vector.tensor_mul(usq[:], uT[:, uslc(k)], uT[:, uslc(k)])
        nc.tensor.matmul(ss_ps[:], onescol[:], usq[:],
                         start=(k == 0), stop=(k == CK - 1))
    rms = wk.tile([1, TOKC], F32, tag="rms", name="rms")
    nc.scalar.activation(rms[:], ss_ps[:], AF.Sqrt, bias=epsb[:], scale=sclb[:])
    invr = wk.tile([1, TOKC], F32R, tag="invr", name="invr")
    with nc.allow_low_precision(reason="invrms feeds a f32r matmul"):
        nc.vector.reciprocal(invr[:], rms[:])

    # ---- normalized activations: xn[k] = uT[k] * invrep (g folded into W1
    # on the host, so shared/routed share one normalized activation set) ----
    ir_ps = pp.tile([128, TOKC], F32, tag="pp", name="ir_ps")
    nc.tensor.matmul(ir_ps[:], onesrow[:], invr[:], start=True, stop=True)
    xns = []
    for k in range(CK):
        xs = io.tile([128, TOKC], BF16, tag=f"xns{k}", name=f"xns{k}")
        nc.vector.tensor_tensor(xs[:], uT[:, uslc(k)], ir_ps[:], AluOpType.mult)
        xns.append(xs)

    # ---- top-2 sigmoid gating ----
    gT = wk.tile([E_R, TOKC], F32R, tag="gT", name="gT")
    for tt in range(TOKC // 128):
        sc_ps = pp.tile([128, E_R], F32, tag="pp", name=f"sc_ps{tt}")
        for k in range(CK):
            nc.tensor.matmul(
                sc_ps[:], uTr[:, TOKC * k + 128 * tt:TOKC * k + 128 * (tt + 1)],
                cen[:, E_R * k:E_R * (k + 1)],
                start=(k == 0), stop=(k == CK - 1))
        sig = wk.tile([128, E_R], F32, tag="sig", bufs=2, name=f"sig{tt}")
        den = wk.tile([128, 1], F32, tag="den", bufs=2, name=f"den{tt}")
        nc.scalar.activation(sig[:], sc_ps[:], AF.Sigmoid, accum_out=den[:])
        invd = wk.tile([128, 1], F32, tag="invd", bufs=2, name=f"invd{tt}")
        nc.vector.reciprocal(invd[:], den[:])
        m1 = wk.tile([128, 1], F32, tag="m1", bufs=2, name=f"m1_{tt}")
        nc.vector.reduce_max(m1[:], sig[:], axis=AX.X)
        mk1 = wk.tile([128, E_R], F32, tag="mk1", bufs=2, name=f"mk1_{tt}")
        nc.vector.tensor_scalar(mk1[:], sig[:], m1[:], None, AluOpType.is_ge)
        s2 = wk.tile([128, E_R], F32, tag="s2", bufs=2, name=f"s2_{tt}")
        nc.vector.scalar_tensor_tensor(s2[:], mk1[:], negb[:], sig[:],
                                       AluOpType.mult, AluOpType.add)
        m2 = wk.tile([128, 1], F32, tag="m2", bufs=2, name=f"m2_{tt}")
        nc.vector.reduce_max(m2[:], s2[:], axis=AX.X)
        mk = wk.tile([128, E_R], F32, tag="mk", bufs=2, name=f"mk_{tt}")
        nc.vector.tensor_scalar(mk[:], sig[:], m2[:], None, AluOpType.is_ge)
        gsel = wk.tile([128, E_R], F32, tag="gsel", bufs=2, name=f"gsel{tt}")
        nc.vector.tensor_mul(gsel[:], sig[:], mk[:])
        gt8 = wk.tile([128, E_R], F32, tag="gt8", bufs=2, name=f"gt8_{tt}")
        nc.vector.tensor_scalar_mul(gt8[:], gsel[:], invd[:])
        tr_ps = pp.tile([E_R, 128], F32, tag="pp", name=f"tr_ps{tt}")
        nc.tensor.transpose(tr_ps[:], gt8[:], ident[:])
        nc.vector.tensor_copy(gT[:, 128 * tt:128 * (tt + 1)], tr_ps[:])

    # select + broadcast this core's routed-expert gates: one matmul per
    # expert with a one-hot row matrix [E_R, 128] as the stationary side.
    wrep = []
    for j in range(4):
        wr_ps = pp.tile([128, TOKC], F32, tag="pp", name=f"wr_ps{j}")
        nc.tensor.matmul(wr_ps[:], esel[:, 128 * j:128 * (j + 1)], gT[:],
                         start=True, stop=True)
        wr = io.tile([128, TOKC], F32, tag=f"wrep{j}", name=f"wrep{j}")
        nc.vector.tensor_copy(wr[:], wr_ps[:])
        wrep.append(wr)

    # ---- accumulator (one tile, c-tile slices) ----
    acc = io.tile([128, CK * TOKC], F32, tag="acc", name="acc")

    # ---- expert passes ----
    for e in range(NE):
        xn = xns
        b2e = wk.tile([1, CK * 128], F32R, tag="b2e", bufs=2, name=f"b2e{e}")
        nc.sync.dma_start(b2e[:], d["b2r"][:, e * CK * 128:(e + 1) * CK * 128])
        h_act = []
        for mc in range(NMC):
            w1c = w1p.tile([128, CK * 256], BF16, tag="w1c", name=f"w1c_{e}_{mc}")
            half = CK * 256 // 2
            nc.sync.dma_start(w1c[:, :half], d["w1"][e, mc][:, :half])
            nc.sync.dma_start(w1c[:, half:], d["w1"][e, mc][:, half:])
            ph = [pp.tile([128, TOKC], F32, tag="pp", name=f"ph_{e}_{mc}_{m}")
                  for m in range(2)]
            for k in range(CK):
                for m in range(2):
                    nc.tensor.matmul(
                        ph[m][:], w1c[:, 256 * k + 128 * m:256 * k + 128 * (m + 1)],
                        xn[k][:], start=(k == 0), stop=(k == CK - 1))
            for m in range(2):
                hh = 2 * mc + m
                ht = hp.tile([128, TOKC], BF16, tag="h", name=f"h_{e}_{hh}")
                nc.scalar.activation(ht[:], ph[m][:], AF.Gelu,
                                     bias=b1[:, e * HK + hh:e * HK + hh + 1])
                h_act.append(ht)
        py = [pp.tile([128, TOKC], F32, tag="pp", name=f"py_{e}_{m}")
              for m in range(CK)]
        # b2 bias seeds each accumulation group via a K=1 one-hot matmul
        for m in range(CK):
            nc.tensor.matmul(py[m][:], b2e[:, m * 128:(m + 1) * 128], ones512[:],
                             start=True, stop=False)
        for kk in range(HK // 2):
            w2s = w2p.tile([128, 2 * 1024], BF16, tag="w2s", name=f"w2s_{e}_{kk}")
            nc.sync.dma_start(w2s[:, :1024], d["w2"][e, kk][:, :1024])
            nc.sync.dma_start(w2s[:, 1024:], d["w2"][e, kk][:, 1024:])
            for k2 in range(2):
                for m in range(CK):
                    nc.tensor.matmul(
                        py[m][:],
                        w2s[:, 1024 * k2 + 128 * m:1024 * k2 + 128 * (m + 1)],
                        h_act[2 * kk + k2][:], start=False,
                        stop=(kk == HK // 2 - 1 and k2 == 1))
        for m in range(CK):
            aslc = acc[:, uslc(m)]
            if e == 0:
                nc.vector.tensor_copy(aslc, py[m][:])
            else:
                nc.vector.tensor_tensor(py[m][:], py[m][:], wrep[e - 1][:],
                                        AluOpType.mult)
                nc.vector.tensor_add(aslc, aslc, py[m][:])

    # ---- store (single DMA) ----
    nc.sync.dma_start(d["out"], acc[:])


def _prep_inputs(u, g_shared, W1_s, b1_s, W2_s, b2_s,
                 g_routed, W1_r, b1_r, W2_r, b2_r, centroids):
    f = np.float32
    u2 = np.ascontiguousarray(np.asarray(u, f).reshape(B * T, C))
    cenT = np.ascontiguousarray(
        np.asarray(centroids, f).reshape(CK, 128, E_R).transpose(1, 0, 2)
    ).reshape(128, CK * E_R)
    gsh = np.asarray(g_shared, f).reshape(C, 1)
    grt = np.asarray(g_routed, f).reshape(C, 1)

    in_maps = []
    group_cache = {}
    for c in range(NCORES):
        g, q = c // 4, c % 4
        if g not in group_cache:
            W1c = np.concatenate(
                [np.asarray(W1_s[g:g + 1], f) * gsh[None],
                 np.asarray(W1_r[4 * g:4 * g + 4], f) * grt[None]], axis=0)
            W2c = np.concatenate([np.asarray(W2_s[g:g + 1], f),
                                  np.asarray(W2_r[4 * g:4 * g + 4], f)], axis=0)
            b1c = np.concatenate([np.asarray(b1_s[g:g + 1], f),
                                  np.asarray(b1_r[4 * g:4 * g + 4], f)], axis=0)
            b2c = np.concatenate([np.asarray(b2_s[g:g + 1], f),
                                  np.asarray(b2_r[4 * g:4 * g + 4], f)], axis=0)
            bf = mybir.dt.np(BF16)
            # [NE, NMC, 128, CK*256]: W1c[e][128k+p, 256mc+j] -> [e, mc, p, (k j)]
            w1h = np.ascontiguousarray(
                W1c.reshape(NE, CK, 128, NMC, 256).transpose(0, 3, 2, 1, 4)
            ).reshape(NE, NMC, 128, CK * 256).astype(bf)
            # [NE, 16, 128, 2*1024]: W2c[e][128(2kk+k2)+p, c] -> [e, kk, p, (k2 c)]
            w2h = np.ascontiguousarray(
                W2c.reshape(NE, HK // 2, 2, 128, 1024).transpose(0, 1, 3, 2, 4)
            ).reshape(NE, HK // 2, 128, 2 * 1024).astype(bf)
            # [128, NE*HK]: b1all[p, e*HK+hh] = b1[e, 128hh+p]
            b1t = np.ascontiguousarray(
                b1c.reshape(NE, HK, 128).transpose(2, 0, 1)).reshape(128, NE * HK)
            b2rw = np.ascontiguousarray(b2c.reshape(NE, CK, 128)).reshape(1, -1)
            es = np.zeros((E_R, 4, 128), f)
            for j in range(4):
                es[4 * g + j, j, :] = 1.0
            es = es.reshape(E_R, 4 * 128)
            group_cache[g] = (w1h, w2h, b1t, b2rw, es)
        w1h, w2h, b1t, b2rw, es = group_cache[g]
        # [128, CK*TOKC]: uTq[p, 512k+t] = u2[512q+t, 128k+p]
        uTq = np.ascontiguousarray(
            u2[TOKC * q:TOKC * (q + 1)].T.reshape(CK, 128, TOKC).transpose(1, 0, 2)
        ).reshape(128, CK * TOKC)
        in_maps.append({
            "uT": uTq, "uTr": uTq,
            "w1": w1h, "w2": w2h, "b1t": b1t, "b2r": b2rw,
            "cen": cenT, "esel": es,
        })
    return in_maps, u2


def _run(in_maps, trace=False):
    if "nc" not in _CACHE:
        _CACHE["nc"] = _build_program()
    nc = _CACHE["nc"]
    res = bass_utils.run_bass_kernel_spmd(
        nc, in_maps, core_ids=list(range(NCORES)), trace=trace)
    return res


def kernel(**inputs):
    in_maps, u2 = _prep_inputs(**inputs)
    trace = bool(int(os.environ.get("MOE_TRACE", "0")))
    res = _run(in_maps, trace=trace)
    _CACHE["last_results"] = res
    out2 = np.empty((B * T, C), np.float32)
    for q in range(4):
        part = (res.results[q]["outT"] + res.results[q + 4]["outT"])
        part = part.reshape(128, CK, TOKC).transpose(1, 0, 2).reshape(C, TOKC)
        out2[TOKC * q:TOKC * (q + 1)] = part.T + u2[TOKC * q:TOKC * (q + 1)]
    return out2.reshape(B, T, C)
